# revision 10
# baseline (speedup 1.0000x reference)
"""Trainium2 Bass kernel for nn_Gudi_UpProj_Block (dense_cnn).

Reference computation (per batch of 8 samples):
    xu  = zero-stuffed 2x upsample of x  (value at even (h,w), zero elsewhere)
    h   = relu(BN(conv5x5(xu, w1)))      # BN: training-mode batch stats
    o2  = BN(conv3x3(h, w2))
    sc  = BN(conv5x5(xu, wsc))
    out = relu(o2 + sc)

Strategy (v3):
  - Data-parallel over batch: 8 cores x 1 sample.
  - conv5x5 on the zero-stuffed input decomposed into 4 output-parity
    classes (9/6/6/4 taps) -> 4x FLOP reduction; implicit-GEMM matmuls.
  - All matmul operands bf16 (full-rate PE + fast weight load, half the
    DMA bytes); PSUM/stats/output fp32. End-to-end error ~3.5e-3.
  - Collective timing reality (measured): the first collective cannot
    complete before ~70-90us regardless of when it is submitted (launch
    skew + CC bring-up absorb the difference), and each mesh AllGather
    takes ~15us. So: a throwaway warm-up AllGather absorbs the wall;
    AR1 carries conv1 AND shortcut stats (its submit at ~60us is still
    ahead of the wall); AR2 carries only conv2 stats. The shortcut
    plane's affine (ssc*scp+tsc) is precomputed during conv2 so the
    post-AR2 tail is just scale+add / relu / DMA per row-chunk.
  - conv1 runs "k-split" (all ci-chunk-0 taps of all 8 PSUM groups
    first) so the first matmul needs only the first half of x/w1; x is
    DMA'd contiguously into a staging tile (the padded layout would be
    a 4096-descriptor scatter DMA) and copied into the padded tile by
    the DVE. PE warm-up dummy matmuls run during the DMA window.
  - BN1 is applied to hpad in 8 row-chunks on Scalar; conv2's row-groups
    depend only on the chunks they read, so conv2 ramps immediately.
  - Collective gather-in is 8 parallel per-core DMAs (core-major SBUF
    layout), then one strided reduce.
"""

import numpy as np
import ml_dtypes

import concourse.bass as bass
import concourse.bacc as bacc
import concourse.tile as tile
from concourse import mybir
from concourse import bass_utils

F32 = mybir.dt.float32
BF16 = mybir.dt.bfloat16
ACTF = mybir.ActivationFunctionType
ALU = mybir.AluOpType
AX = mybir.AxisListType

N_CORES = 8
EPS = 1e-5
N_NORM = 8 * 64 * 64  # BN count over (N, H, W)

PARITIES = [(0, 0), (0, 1), (1, 0), (1, 1)]


def _taps5(r, s):
    iis = (0, 2, 4) if r == 0 else (1, 3)
    jjs = (0, 2, 4) if s == 0 else (1, 3)
    return [(i, j) for i in iis for j in jjs]


def _build_program(nc):
    xs_d = nc.dram_tensor("xs", [256, 32, 32], BF16, kind="ExternalInput").ap()
    w1t_d = nc.dram_tensor("w1t", [256, 3200], BF16, kind="ExternalInput").ap()
    wsct_d = nc.dram_tensor("wsct", [256, 3200], BF16, kind="ExternalInput").ap()
    w2t_d = nc.dram_tensor("w2t", [128, 1152], BF16, kind="ExternalInput").ap()
    bnp_d = nc.dram_tensor("bnp", [128, 6], F32, kind="ExternalInput").ap()
    out_d = nc.dram_tensor("out", [128, 64, 64], F32, kind="ExternalOutput").ap()

    with tile.TileContext(nc) as tc:
        with (
            tc.tile_pool(name="consts", bufs=1) as consts,
            tc.tile_pool(name="psum", bufs=8, space="PSUM") as psum,
            tc.tile_pool(name="scratch", bufs=2) as scratch,
            tc.tile_pool(name="fin", bufs=3) as finp,
            tc.tile_pool(name="dram", bufs=1, space="DRAM") as dram,
        ):
            # ---- persistent SBUF tiles ----
            xstage = [consts.tile([128, 32, 32], BF16, name=f"xstage{k}",
                                  tag=f"xstage{k}") for k in range(2)]
            xpad = [consts.tile([128, 34, 34], BF16, name=f"xpad{k}",
                                tag=f"xpad{k}") for k in range(2)]
            w1sb = [consts.tile([128, 3200], BF16, name=f"w1sb{k}",
                                tag=f"w1sb{k}") for k in range(2)]
            wscsb = [consts.tile([128, 3200], BF16, name=f"wscsb{k}",
                                 tag=f"wscsb{k}") for k in range(2)]
            w2sb = consts.tile([128, 1152], BF16, name="w2sb", tag="w2sb")
            bnp = consts.tile([128, 6], F32, name="bnp_sb", tag="bnp_sb")
            hpad = consts.tile([128, 66, 66], BF16, name="hpad", tag="hpad")
            scp = consts.tile([128, 64, 64], F32, name="scp", tag="scp")
            sapl = consts.tile([128, 64, 64], F32, name="sapl", tag="sapl")
            # stat tiles: cols 0..7 = per-group sums, 8..15 = sum-of-squares
            st1 = consts.tile([128, 16], F32, name="st1", tag="st1")
            stsc = consts.tile([128, 16], F32, name="stsc", tag="stsc")
            st2 = consts.tile([128, 16], F32, name="st2", tag="st2")
            arA_sb = consts.tile([128, 4], F32, name="arA_sb", tag="arA_sb")
            arA_g = consts.tile([128, 32], F32, name="arA_g", tag="arA_g")
            arA_res = consts.tile([128, 4], F32, name="arA_res", tag="arA_res")
            arB_sb = consts.tile([128, 2], F32, name="arB_sb", tag="arB_sb")
            arB_g = consts.tile([128, 16], F32, name="arB_g", tag="arB_g")
            arB_res = consts.tile([128, 2], F32, name="arB_res", tag="arB_res")
            coef = consts.tile([128, 32], F32, name="coef", tag="coef")
            zb = consts.tile([128, 128], BF16, name="zb", tag="zb")
            zflat = consts.tile([128, 132], BF16, name="zflat", tag="zflat")
            zf32 = consts.tile([128, 4], F32, name="zf32", tag="zf32")

            nc.vector.memset(zflat[:], 0.0)
            nc.vector.memset(zb[:], 0.0)
            nc.vector.memset(zf32[:], 0.0)

            # Dummy Sqrt as the very first Act op: forces the act-table pass
            # to load a sqrt-bearing set (covers Copy/Square/Relu too) during
            # the input-DMA window instead of mid-kernel (~3.6us swap).
            nc.scalar.activation(coef[:, 31:32], zf32[:, 0:1], ACTF.Sqrt)
            eps_col = coef[:, 30:31]
            nc.vector.memset(eps_col, EPS)

            # ---- DRAM scratch for collectives ----
            arW_in_d = dram.tile([128, 1], F32, name="arW_in_d", tag="arW_in_d")
            arW_g_d = dram.tile([8, 128, 1], F32, name="arW_g_d", tag="arW_g_d")
            arA_in_d = dram.tile([128, 4], F32, name="arA_in_d", tag="arA_in_d")
            arA_g_d = dram.tile([8, 128, 4], F32, name="arA_g_d", tag="arA_g_d")
            arB_in_d = dram.tile([128, 2], F32, name="arB_in_d", tag="arB_in_d")
            arB_g_d = dram.tile([8, 128, 2], F32, name="arB_g_d", tag="arB_g_d")

            # ---- input DMAs: w1sb[0] + xstage[0] gate the first matmul.
            # x goes through a contiguous staging tile (the padded layout
            # would be a 4096-descriptor DMA that stalls the queue).
            nc.sync.dma_start(w1sb[0][:], w1t_d[0:128, :])
            nc.sync.dma_start(xstage[0][:], xs_d[0:128, :, :])
            nc.sync.dma_start(w1sb[1][:], w1t_d[128:256, :])
            nc.sync.dma_start(xstage[1][:], xs_d[128:256, :, :])
            for k in range(2):
                nc.sync.dma_start(wscsb[k][:], wsct_d[k * 128:(k + 1) * 128, :])
            nc.sync.dma_start(w2sb[:], w2t_d[:])
            nc.sync.dma_start(bnp[:], bnp_d[:])

            # ---- warm-up collective: absorbs launch skew + CC bring-up so
            # the two real collectives run at their latency floor.
            nc.sync.dma_start(arW_in_d[:], zf32[:, 0:1])
            nc.gpsimd.collective_compute(
                "AllGather", ALU.bypass,
                ins=[arW_in_d.opt()], outs=[arW_g_d.opt()],
                replica_groups=[list(range(N_CORES))],
            )

            # ---- PE warm-up: dummy matmuls during the DMA wait flip the
            # HAM clock gate to 2.4 GHz before conv1 arrives.
            pdum = psum.tile([128, 16, 32], F32, tag="pbank", name="pdum")
            zbv = zb[:].rearrange("p (a b) -> p a b", a=4)
            for i in range(30):
                nc.tensor.matmul(pdum[:, 0:4, :], zb[:], zbv,
                                 start=True, stop=True)

            # xpad borders + interior copies (DVE; disjoint regions)
            for k in range(2):
                nc.vector.tensor_copy(xpad[k][:, 0, :], zflat[:, 0:34])
                nc.vector.tensor_copy(xpad[k][:, 33, :], zflat[:, 0:34])
                nc.vector.tensor_copy(xpad[k][:, 1:33, 0], zflat[:, 0:32])
                nc.vector.tensor_copy(xpad[k][:, 1:33, 33], zflat[:, 0:32])
                nc.vector.tensor_copy(xpad[k][:, 1:33, 1:33], xstage[k][:])

            # hpad border zeros (interior is fully written by conv1 scatter)
            nc.vector.tensor_copy(hpad[:, 0, :], zflat[:, 0:66])
            nc.vector.tensor_copy(hpad[:, 65, :], zflat[:, 0:66])
            nc.vector.tensor_copy(hpad[:, 1:65, 0], zflat[:, 0:64])
            nc.vector.tensor_copy(hpad[:, 1:65, 65], zflat[:, 0:64])

            def conv5_groups(wsb, scatter_to_hpad, stt):
                """Zero-stuffed 5x5 conv: 8 PSUM groups (4 parities x 2 row
                halves), k-split: all chunk-0 taps of all groups first so the
                first matmul only needs the first half of the inputs.
                Drains: Act Copy + Act Square on Scalar, accum stats."""
                groups = []
                for half in range(2):
                    for (r, s) in PARITIES:
                        pt = psum.tile([128, 16, 32], F32, tag="pbank",
                                       name=f"pb_{id(wsb)}_{half}_{r}{s}")
                        groups.append((pt, half, r, s))
                for k in range(2):
                    for gi, (pt, half, r, s) in enumerate(groups):
                        taps = _taps5(r, s)
                        for idx, (i, j) in enumerate(taps):
                            di = (r - 2 + i) // 2
                            dj = (s - 2 + j) // 2
                            tapn = 5 * i + j
                            r0 = 1 + 16 * half + di
                            c0 = 1 + dj
                            nc.tensor.matmul(
                                pt[:],
                                wsb[k][:, 128 * tapn:128 * tapn + 128],
                                xpad[k][:, r0:r0 + 16, c0:c0 + 32],
                                start=(k == 0 and idx == 0),
                                stop=(k == 1 and idx == len(taps) - 1),
                            )
                        if k == 1:
                            if scatter_to_hpad:
                                dst = hpad[:, 1 + r + 32 * half:
                                           1 + r + 32 * half + 32:2,
                                           1 + s:1 + s + 64:2]
                            else:
                                dst = scp[:, r + 32 * half:32 * half + 32:2,
                                          s:64:2]
                            nc.scalar.activation(dst, pt[:], ACTF.Copy,
                                                 accum_out=stt[:, gi:gi + 1])
                            sq = scratch.tile([128, 16, 32], F32, tag="sq",
                                              name="sq")
                            nc.scalar.activation(
                                sq[:], pt[:], ACTF.Square,
                                accum_out=stt[:, 8 + gi:9 + gi])

            # ---- conv1 then shortcut conv (PE back-to-back) ----
            conv5_groups(w1sb, True, st1)
            conv5_groups(wscsb, False, stsc)

            # ---- AllGather #1: conv1 + shortcut stats [S1,Q1,Ssc,Qsc] ----
            nc.vector.reduce_sum(
                out=arA_sb[:, 0:2],
                in_=st1[:].rearrange("p (s g) -> p s g", s=2), axis=AX.X)
            nc.vector.reduce_sum(
                out=arA_sb[:, 2:4],
                in_=stsc[:].rearrange("p (s g) -> p s g", s=2), axis=AX.X)
            nc.sync.dma_start(arA_in_d[:], arA_sb[:])
            nc.gpsimd.collective_compute(
                "AllGather", ALU.bypass,
                ins=[arA_in_d.opt()], outs=[arA_g_d.opt()],
                replica_groups=[list(range(N_CORES))],
            )
            # gathered -> SBUF: 8 parallel per-core DMAs, core-major layout
            for c in range(N_CORES):
                nc.sync.dma_start(arA_g[:, 4 * c:4 * c + 4], arA_g_d[c])
            nc.vector.reduce_sum(
                out=arA_res[:, 0:4],
                in_=arA_g[:].rearrange("p (c s) -> p s c", c=8), axis=AX.X)

            # ---- BN1 + BNsc coefficients (pairs [conv1, shortcut]) ----
            inv_n = 1.0 / float(N_NORM)
            meA = coef[:, 0:4]   # [m1, msc, q1, qsc]
            nc.vector.tensor_scalar(
                out=coef[:, 0:2], in0=arA_res[:, 0:4:2], scalar1=inv_n,
                scalar2=None, op0=ALU.mult)
            nc.vector.tensor_scalar(
                out=coef[:, 2:4], in0=arA_res[:, 1:4:2], scalar1=inv_n,
                scalar2=None, op0=ALU.mult)
            msqA = coef[:, 4:6]
            nc.vector.scalar_tensor_tensor(
                out=msqA, in0=coef[:, 0:2], scalar=1.0, in1=coef[:, 0:2],
                op0=ALU.mult, op1=ALU.mult)
            varA = coef[:, 6:8]
            nc.vector.tensor_sub(varA, coef[:, 2:4], msqA)
            sdA = coef[:, 8:10]
            nc.scalar.activation(sdA, varA, ACTF.Sqrt, bias=eps_col)
            rstdA = coef[:, 10:12]
            nc.vector.reciprocal(rstdA, sdA)
            sA = coef[:, 12:14]   # [s1, ssc]
            nc.vector.tensor_mul(sA, bnp[:, 0:3:2], rstdA)
            msA = coef[:, 14:16]
            nc.vector.tensor_mul(msA, coef[:, 0:2], sA)
            tA = coef[:, 16:18]   # [t1, tsc]
            nc.vector.tensor_sub(tA, bnp[:, 1:4:2], msA)
            s1_ap, ssc_ap = sA[:, 0:1], sA[:, 1:2]
            t1_ap, tsc_ap = tA[:, 0:1], tA[:, 1:2]

            # ---- BN1 + ReLU on hpad, 8 row-chunks (pipelines into conv2) ----
            for c in range(8):
                nc.scalar.activation(
                    hpad[:, 1 + 8 * c:9 + 8 * c, 1:65],
                    hpad[:, 1 + 8 * c:9 + 8 * c, 1:65],
                    ACTF.Relu, bias=t1_ap, scale=s1_ap)

            # ---- shortcut plane affine: sapl = ssc*scp + tsc (one GpSimd
            # op; runs during conv2, so the post-AR2 tail never touches it)
            nc.gpsimd.tensor_scalar(
                out=sapl[:], in0=scp[:], scalar1=ssc_ap, scalar2=tsc_ap,
                op0=ALU.mult, op1=ALU.add)

            # ---- conv2 (3x3 over h) ----
            p2s = []
            for c in range(8):
                pt2 = psum.tile([128, 8, 64], F32, tag="pbank", name=f"p2_{c}")
                for idx, (i, j) in enumerate(
                        [(i, j) for i in range(3) for j in range(3)]):
                    di, dj = i - 1, j - 1
                    tapn = 3 * i + j
                    nc.tensor.matmul(
                        pt2[:],
                        w2sb[:, 128 * tapn:128 * tapn + 128],
                        hpad[:, 1 + 8 * c + di:1 + 8 * c + di + 8,
                             1 + dj:1 + dj + 64],
                        start=(idx == 0),
                        stop=(idx == 8),
                    )
                p2s.append(pt2)
                nc.vector.reduce_sum(out=st2[:, c:c + 1], in_=pt2[:], axis=AX.XY)
                sq2 = scratch.tile([128, 8, 64], F32, tag="sq", name="sq2")
                nc.scalar.activation(sq2[:], pt2[:], ACTF.Square,
                                     accum_out=st2[:, 8 + c:9 + c])

            # ---- AllGather #2: conv2 stats [S2,Q2] ----
            nc.vector.reduce_sum(
                out=arB_sb[:, 0:2],
                in_=st2[:].rearrange("p (s g) -> p s g", s=2), axis=AX.X)
            nc.sync.dma_start(arB_in_d[:], arB_sb[:])
            nc.gpsimd.collective_compute(
                "AllGather", ALU.bypass,
                ins=[arB_in_d.opt()], outs=[arB_g_d.opt()],
                replica_groups=[list(range(N_CORES))],
            )
            for c in range(N_CORES):
                nc.sync.dma_start(arB_g[:, 2 * c:2 * c + 2], arB_g_d[c])
            nc.vector.reduce_sum(
                out=arB_res[:, 0:2],
                in_=arB_g[:].rearrange("p (c s) -> p s c", c=8), axis=AX.X)

            # ---- BN2 coefficients ----
            meB = coef[:, 20:22]  # [m2, q2]
            nc.vector.tensor_scalar(
                out=meB, in0=arB_res[:, 0:2], scalar1=inv_n, scalar2=None,
                op0=ALU.mult)
            msqB = coef[:, 22:23]
            nc.vector.scalar_tensor_tensor(
                out=msqB, in0=meB[:, 0:1], scalar=1.0, in1=meB[:, 0:1],
                op0=ALU.mult, op1=ALU.mult)
            varB = coef[:, 23:24]
            nc.vector.tensor_sub(varB, meB[:, 1:2], msqB)
            sdB = coef[:, 24:25]
            nc.scalar.activation(sdB, varB, ACTF.Sqrt, bias=eps_col)
            rstdB = coef[:, 25:26]
            nc.vector.reciprocal(rstdB, sdB)
            s2_ap = coef[:, 26:27]
            nc.vector.tensor_mul(s2_ap, bnp[:, 4:5], rstdB)
            msB = coef[:, 27:28]
            nc.vector.tensor_mul(msB, meB[:, 0:1], s2_ap)
            t2_ap = coef[:, 28:29]
            nc.vector.tensor_sub(t2_ap, bnp[:, 5:6], msB)

            # ---- final: out = relu(s2*conv2 + sapl + t2), chunked ----
            for c in range(8):
                fin = finp.tile([128, 8, 64], F32, tag="fin", name="fin")
                nc.vector.scalar_tensor_tensor(
                    out=fin[:], in0=p2s[c][:], scalar=s2_ap,
                    in1=sapl[:, 8 * c:8 * c + 8, :],
                    op0=ALU.mult, op1=ALU.add,
                )
                ob = finp.tile([128, 8, 64], F32, tag="ob", name="ob")
                nc.scalar.activation(ob[:], fin[:], ACTF.Relu, bias=t2_ap)
                nc.sync.dma_start(out_d[:, 8 * c:8 * c + 8, :], ob[:])

    return nc


_CACHE = {}

# Set by test harness: run with trace=True and stash profiling info here.
TRACE = False
LAST = {}


def _get_nc():
    if "nc" not in _CACHE:
        nc = bacc.Bacc("TRN2", target_bir_lowering=False, debug=False,
                       num_devices=N_CORES)
        _build_program(nc)
        nc.compile()
        _CACHE["nc"] = nc
    return _CACHE["nc"]


def _pack_inputs(x, w1, g1, b1, w2, g2, b2, wsc, gsc, bsc):
    bf = ml_dtypes.bfloat16
    w1t = np.ascontiguousarray(
        w1.transpose(1, 2, 3, 0).reshape(256, 3200).astype(bf))
    wsct = np.ascontiguousarray(
        wsc.transpose(1, 2, 3, 0).reshape(256, 3200).astype(bf))
    w2t = np.ascontiguousarray(
        w2.transpose(1, 2, 3, 0).reshape(128, 1152).astype(bf))
    bnp = np.ascontiguousarray(
        np.stack([g1, b1, gsc, bsc, g2, b2], axis=1), dtype=np.float32)
    xb = x.astype(bf)
    in_maps = []
    for c in range(N_CORES):
        in_maps.append({
            "xs": np.ascontiguousarray(xb[c]),
            "w1t": w1t,
            "wsct": wsct,
            "w2t": w2t,
            "bnp": bnp,
        })
    return in_maps


def kernel(x, w1, g1, b1, w2, g2, b2, wsc, gsc, bsc):
    nc = _get_nc()
    in_maps = _pack_inputs(x, w1, g1, b1, w2, g2, b2, wsc, gsc, bsc)
    res = bass_utils.run_bass_kernel_spmd(
        nc, in_maps, core_ids=list(range(N_CORES)), trace=TRACE,
    )
    LAST["exec_time_ns"] = res.exec_time_ns
    LAST["results"] = res
    out = np.stack([res.results[c]["out"] for c in range(N_CORES)], axis=0)
    return out.astype(np.float32)


# revision 14
# speedup vs baseline: 1.1300x; 1.1300x over previous
"""Trainium2 Bass kernel for nn_Gudi_UpProj_Block (dense_cnn).

Reference computation (per batch of 8 samples):
    xu  = zero-stuffed 2x upsample of x  (value at even (h,w), zero elsewhere)
    h   = relu(BN(conv5x5(xu, w1)))      # BN: training-mode batch stats
    o2  = BN(conv3x3(h, w2))
    sc  = BN(conv5x5(xu, wsc))
    out = relu(o2 + sc)

Strategy (v3):
  - Data-parallel over batch: 8 cores x 1 sample.
  - conv5x5 on the zero-stuffed input decomposed into 4 output-parity
    classes (9/6/6/4 taps) -> 4x FLOP reduction; implicit-GEMM matmuls.
  - All matmul operands bf16 (full-rate PE + fast weight load, half the
    DMA bytes); PSUM/stats/output fp32. End-to-end error ~3.5e-3.
  - Collective timing reality (measured): the first collective cannot
    complete before ~70-90us regardless of when it is submitted (launch
    skew + CC bring-up absorb the difference), and each mesh AllGather
    takes ~15us. So: a throwaway warm-up AllGather absorbs the wall;
    AR1 carries conv1 AND shortcut stats (its submit at ~60us is still
    ahead of the wall); AR2 carries only conv2 stats. The shortcut
    plane's affine (ssc*scp+tsc) is precomputed during conv2 so the
    post-AR2 tail is just scale+add / relu / DMA per row-chunk.
  - conv1 runs "k-split" (all ci-chunk-0 taps of all 8 PSUM groups
    first) so the first matmul needs only the first half of x/w1; x is
    DMA'd contiguously into a staging tile (the padded layout would be
    a 4096-descriptor scatter DMA) and copied into the padded tile by
    the DVE. PE warm-up dummy matmuls run during the DMA window.
  - BN1 is applied to hpad in 8 row-chunks on Scalar; conv2's row-groups
    depend only on the chunks they read, so conv2 ramps immediately.
  - Collective gather-in is 8 parallel per-core DMAs (core-major SBUF
    layout), then one strided reduce.
"""

import numpy as np
import ml_dtypes

import concourse.bass as bass
import concourse.bacc as bacc
import concourse.tile as tile
from concourse import mybir
from concourse import bass_utils

F32 = mybir.dt.float32
BF16 = mybir.dt.bfloat16
ACTF = mybir.ActivationFunctionType
ALU = mybir.AluOpType
AX = mybir.AxisListType

N_CORES = 8
EPS = 1e-5
N_NORM = 8 * 64 * 64  # BN count over (N, H, W)

PARITIES = [(0, 0), (0, 1), (1, 0), (1, 1)]


def _taps5(r, s):
    iis = (0, 2, 4) if r == 0 else (1, 3)
    jjs = (0, 2, 4) if s == 0 else (1, 3)
    return [(i, j) for i in iis for j in jjs]


def _build_program(nc):
    xs_d = nc.dram_tensor("xs", [256, 32, 32], BF16, kind="ExternalInput").ap()
    w1t_d = nc.dram_tensor("w1t", [256, 3200], BF16, kind="ExternalInput").ap()
    wsct_d = nc.dram_tensor("wsct", [256, 3200], BF16, kind="ExternalInput").ap()
    w2t_d = nc.dram_tensor("w2t", [128, 1152], BF16, kind="ExternalInput").ap()
    bnp_d = nc.dram_tensor("bnp", [128, 6], F32, kind="ExternalInput").ap()
    out_d = nc.dram_tensor("out", [128, 64, 64], F32, kind="ExternalOutput").ap()

    with tile.TileContext(nc) as tc:
        with (
            tc.tile_pool(name="consts", bufs=1) as consts,
            tc.tile_pool(name="psum", bufs=8, space="PSUM") as psum,
            tc.tile_pool(name="scratch", bufs=2) as scratch,
            tc.tile_pool(name="fin", bufs=3) as finp,
            tc.tile_pool(name="dram", bufs=1, space="DRAM") as dram,
        ):
            # ---- persistent SBUF tiles ----
            xstage = [consts.tile([128, 32, 32], BF16, name=f"xstage{k}",
                                  tag=f"xstage{k}") for k in range(2)]
            xpad = [consts.tile([128, 34, 34], BF16, name=f"xpad{k}",
                                tag=f"xpad{k}") for k in range(2)]
            w1sb = [consts.tile([128, 3200], BF16, name=f"w1sb{k}",
                                tag=f"w1sb{k}") for k in range(2)]
            wscsb = [consts.tile([128, 3200], BF16, name=f"wscsb{k}",
                                 tag=f"wscsb{k}") for k in range(2)]
            w2sb = consts.tile([128, 1152], BF16, name="w2sb", tag="w2sb")
            bnp = consts.tile([128, 6], F32, name="bnp_sb", tag="bnp_sb")
            hpad = consts.tile([128, 66, 66], BF16, name="hpad", tag="hpad")
            scp = consts.tile([128, 64, 64], F32, name="scp", tag="scp")
            sapl = consts.tile([128, 64, 64], F32, name="sapl", tag="sapl")
            # stat tiles: cols 0..7 = per-group sums, 8..15 = sum-of-squares
            st1 = consts.tile([128, 16], F32, name="st1", tag="st1")
            stsc = consts.tile([128, 16], F32, name="stsc", tag="stsc")
            st2 = consts.tile([128, 16], F32, name="st2", tag="st2")
            arA_sb = consts.tile([128, 2], F32, name="arA_sb", tag="arA_sb")
            arA_g = consts.tile([128, 16], F32, name="arA_g", tag="arA_g")
            arA_res = consts.tile([128, 2], F32, name="arA_res", tag="arA_res")
            arS_sb = consts.tile([128, 2], F32, name="arS_sb", tag="arS_sb")
            arS_g = consts.tile([128, 16], F32, name="arS_g", tag="arS_g")
            arS_res = consts.tile([128, 2], F32, name="arS_res", tag="arS_res")
            arB_sb = consts.tile([128, 2], F32, name="arB_sb", tag="arB_sb")
            arB_g = consts.tile([128, 16], F32, name="arB_g", tag="arB_g")
            arB_res = consts.tile([128, 2], F32, name="arB_res", tag="arB_res")
            coef = consts.tile([128, 32], F32, name="coef", tag="coef")
            zb = consts.tile([128, 128], BF16, name="zb", tag="zb")
            zflat = consts.tile([128, 132], BF16, name="zflat", tag="zflat")
            zf32 = consts.tile([128, 4], F32, name="zf32", tag="zf32")

            nc.vector.memset(zflat[:], 0.0)
            nc.vector.memset(zb[:], 0.0)
            nc.vector.memset(zf32[:], 0.0)

            # Dummy Sqrt as the very first Act op: forces the act-table pass
            # to load a sqrt-bearing set (covers Copy/Square/Relu too) during
            # the input-DMA window instead of mid-kernel (~3.6us swap).
            nc.scalar.activation(coef[:, 31:32], zf32[:, 0:1], ACTF.Sqrt)
            eps_col = coef[:, 30:31]
            nc.vector.memset(eps_col, EPS)

            # ---- DRAM scratch for collectives ----
            arW_in_d = dram.tile([128, 1], F32, name="arW_in_d", tag="arW_in_d")
            arW_g_d = dram.tile([8, 128, 1], F32, name="arW_g_d", tag="arW_g_d")
            arA_in_d = dram.tile([128, 2], F32, name="arA_in_d", tag="arA_in_d")
            arA_g_d = dram.tile([8, 128, 2], F32, name="arA_g_d", tag="arA_g_d")
            arS_in_d = dram.tile([128, 2], F32, name="arS_in_d", tag="arS_in_d")
            arS_g_d = dram.tile([8, 128, 2], F32, name="arS_g_d", tag="arS_g_d")
            arB_in_d = dram.tile([128, 2], F32, name="arB_in_d", tag="arB_in_d")
            arB_g_d = dram.tile([8, 128, 2], F32, name="arB_g_d", tag="arB_g_d")

            # ---- input DMAs: w1sb[0] + xstage[0] gate the first matmul.
            # x goes through a contiguous staging tile (the padded layout
            # would be a 4096-descriptor DMA that stalls the queue).
            nc.sync.dma_start(w1sb[0][:], w1t_d[0:128, :])
            nc.sync.dma_start(xstage[0][:], xs_d[0:128, :, :])
            nc.sync.dma_start(w1sb[1][:], w1t_d[128:256, :])
            nc.sync.dma_start(xstage[1][:], xs_d[128:256, :, :])
            for k in range(2):
                nc.sync.dma_start(wscsb[k][:], wsct_d[k * 128:(k + 1) * 128, :])
            nc.sync.dma_start(w2sb[:], w2t_d[:])
            nc.sync.dma_start(bnp[:], bnp_d[:])

            # ---- warm-up collective: absorbs launch skew + CC bring-up so
            # the two real collectives run at their latency floor.
            nc.sync.dma_start(arW_in_d[:], zf32[:, 0:1])
            nc.gpsimd.collective_compute(
                "AllGather", ALU.bypass,
                ins=[arW_in_d.opt()], outs=[arW_g_d.opt()],
                replica_groups=[list(range(N_CORES))],
            )

            # ---- PE warm-up: dummy matmuls during the DMA wait flip the
            # HAM clock gate to 2.4 GHz before conv1 arrives.
            pdum = psum.tile([128, 16, 32], F32, tag="pbank", name="pdum")
            zbv = zb[:].rearrange("p (a b) -> p a b", a=4)
            for i in range(30):
                nc.tensor.matmul(pdum[:, 0:4, :], zb[:], zbv,
                                 start=True, stop=True)

            # xpad borders + interior copies (DVE; disjoint regions)
            for k in range(2):
                nc.vector.tensor_copy(xpad[k][:, 0, :], zflat[:, 0:34])
                nc.vector.tensor_copy(xpad[k][:, 33, :], zflat[:, 0:34])
                nc.vector.tensor_copy(xpad[k][:, 1:33, 0], zflat[:, 0:32])
                nc.vector.tensor_copy(xpad[k][:, 1:33, 33], zflat[:, 0:32])
                nc.vector.tensor_copy(xpad[k][:, 1:33, 1:33], xstage[k][:])

            # hpad border zeros (interior is fully written by conv1 scatter)
            nc.vector.tensor_copy(hpad[:, 0, :], zflat[:, 0:66])
            nc.vector.tensor_copy(hpad[:, 65, :], zflat[:, 0:66])
            nc.vector.tensor_copy(hpad[:, 1:65, 0], zflat[:, 0:64])
            nc.vector.tensor_copy(hpad[:, 1:65, 65], zflat[:, 0:64])

            def conv5_groups(wsb, scatter_to_hpad, stt):
                """Zero-stuffed 5x5 conv: 8 PSUM groups (4 parities x 2 row
                halves), k-split: all chunk-0 taps of all groups first so the
                first matmul only needs the first half of the inputs.
                Drains: Act Copy + Act Square on Scalar, accum stats."""
                groups = []
                for half in range(2):
                    for (r, s) in PARITIES:
                        pt = psum.tile([128, 16, 32], F32, tag="pbank",
                                       name=f"pb_{id(wsb)}_{half}_{r}{s}")
                        groups.append((pt, half, r, s))
                for k in range(2):
                    for gi, (pt, half, r, s) in enumerate(groups):
                        taps = _taps5(r, s)
                        for idx, (i, j) in enumerate(taps):
                            di = (r - 2 + i) // 2
                            dj = (s - 2 + j) // 2
                            tapn = 5 * i + j
                            r0 = 1 + 16 * half + di
                            c0 = 1 + dj
                            nc.tensor.matmul(
                                pt[:],
                                wsb[k][:, 128 * tapn:128 * tapn + 128],
                                xpad[k][:, r0:r0 + 16, c0:c0 + 32],
                                start=(k == 0 and idx == 0),
                                stop=(k == 1 and idx == len(taps) - 1),
                            )
                        if k == 1:
                            if scatter_to_hpad:
                                dst = hpad[:, 1 + r + 32 * half:
                                           1 + r + 32 * half + 32:2,
                                           1 + s:1 + s + 64:2]
                            else:
                                dst = scp[:, r + 32 * half:32 * half + 32:2,
                                          s:64:2]
                            nc.scalar.activation(dst, pt[:], ACTF.Copy,
                                                 accum_out=stt[:, gi:gi + 1])
                            sq = scratch.tile([128, 16, 32], F32, tag="sq",
                                              name="sq")
                            nc.scalar.activation(
                                sq[:], pt[:], ACTF.Square,
                                accum_out=stt[:, 8 + gi:9 + gi])

            def emit_bn1(S_Q, G, B, cb):
                """mean/var -> scale s, shift t from summed stats (m=1).
                Returns (s_ap, t_ap) in coef cols cb..cb+7."""
                inv_n = 1.0 / float(N_NORM)
                me = coef[:, cb:cb + 2]      # [mean, ex2]
                nc.vector.tensor_scalar(
                    out=me, in0=S_Q, scalar1=inv_n, scalar2=None, op0=ALU.mult)
                msq = coef[:, cb + 2:cb + 3]
                nc.vector.scalar_tensor_tensor(
                    out=msq, in0=me[:, 0:1], scalar=1.0, in1=me[:, 0:1],
                    op0=ALU.mult, op1=ALU.mult)
                var = coef[:, cb + 3:cb + 4]
                nc.vector.tensor_sub(var, me[:, 1:2], msq)
                sd = coef[:, cb + 4:cb + 5]
                nc.scalar.activation(sd, var, ACTF.Sqrt, bias=eps_col)
                rstd = coef[:, cb + 5:cb + 6]
                nc.vector.reciprocal(rstd, sd)
                s_ = coef[:, cb + 6:cb + 7]
                nc.vector.tensor_mul(s_, G, rstd)
                ms = coef[:, cb + 7:cb + 8]
                nc.vector.tensor_mul(ms, me[:, 0:1], s_)
                t_ = coef[:, cb + 8:cb + 9]
                nc.vector.tensor_sub(t_, B, ms)
                return s_, t_

            def ag_submit(sb, in_d, g_d, stt):
                nc.vector.reduce_sum(
                    out=sb[:, 0:2],
                    in_=stt[:].rearrange("p (s g) -> p s g", s=2), axis=AX.X)
                nc.sync.dma_start(in_d[:], sb[:])
                nc.gpsimd.collective_compute(
                    "AllGather", ALU.bypass,
                    ins=[in_d.opt()], outs=[g_d.opt()],
                    replica_groups=[list(range(N_CORES))],
                )

            def ag_land(g_sb, g_d, res):
                nc.sync.dma_start(
                    g_sb[:].rearrange("p (c s) -> p c s", c=8),
                    g_d[:].rearrange("c p s -> p c s"),
                )
                nc.vector.reduce_sum(
                    out=res[:, 0:2],
                    in_=g_sb[:].rearrange("p (c s) -> p s c", c=8), axis=AX.X)

            # ---- conv1; submit its stats collective immediately ----
            conv5_groups(w1sb, True, st1)
            ag_submit(arA_sb, arA_in_d, arA_g_d, st1)

            # ---- shortcut conv (PE back-to-back); submit its stats ----
            conv5_groups(wscsb, False, stsc)
            ag_submit(arS_sb, arS_in_d, arS_g_d, stsc)

            # ---- AR1 lands: BN1 coefs + BN1+ReLU on hpad in 8 chunks ----
            ag_land(arA_g, arA_g_d, arA_res)
            s1_ap, t1_ap = emit_bn1(arA_res[:, 0:2], bnp[:, 0:1], bnp[:, 1:2], 0)
            for c in range(8):
                nc.scalar.activation(
                    hpad[:, 1 + 8 * c:9 + 8 * c, 1:65],
                    hpad[:, 1 + 8 * c:9 + 8 * c, 1:65],
                    ACTF.Relu, bias=t1_ap, scale=s1_ap)

            # ---- conv2 (3x3 over h) ----
            p2s = []
            for c in range(8):
                pt2 = psum.tile([128, 8, 64], F32, tag="pbank", name=f"p2_{c}")
                for idx, (i, j) in enumerate(
                        [(i, j) for i in range(3) for j in range(3)]):
                    di, dj = i - 1, j - 1
                    tapn = 3 * i + j
                    nc.tensor.matmul(
                        pt2[:],
                        w2sb[:, 128 * tapn:128 * tapn + 128],
                        hpad[:, 1 + 8 * c + di:1 + 8 * c + di + 8,
                             1 + dj:1 + dj + 64],
                        start=(idx == 0),
                        stop=(idx == 8),
                    )
                p2s.append(pt2)
                nc.vector.reduce_sum(out=st2[:, c:c + 1], in_=pt2[:], axis=AX.XY)
                sq2 = scratch.tile([128, 8, 64], F32, tag="sq", name="sq2")
                nc.scalar.activation(sq2[:], pt2[:], ACTF.Square,
                                     accum_out=st2[:, 8 + c:9 + c])

            # ---- submit conv2 stats, then land shortcut stats (order keeps
            # the GpSimd/Vector FIFOs compatible with readiness times)
            ag_submit(arB_sb, arB_in_d, arB_g_d, st2)

            ag_land(arS_g, arS_g_d, arS_res)
            ssc_ap, tsc_ap = emit_bn1(arS_res[:, 0:2], bnp[:, 2:3], bnp[:, 3:4], 10)
            # shortcut plane affine: sapl = ssc*scp + tsc (GpSimd; runs
            # during the AR2 wait, so the post-AR2 tail never touches scp)
            nc.gpsimd.tensor_scalar(
                out=sapl[:], in0=scp[:], scalar1=ssc_ap, scalar2=tsc_ap,
                op0=ALU.mult, op1=ALU.add)

            # ---- AR2 lands: BN2 coefficients ----
            ag_land(arB_g, arB_g_d, arB_res)
            s2_ap, t2_ap = emit_bn1(arB_res[:, 0:2], bnp[:, 4:5], bnp[:, 5:6], 20)

            # ---- final: out = relu(s2*conv2 + sapl + t2), chunked ----
            for c in range(8):
                fin = finp.tile([128, 8, 64], F32, tag="fin", name="fin")
                nc.vector.scalar_tensor_tensor(
                    out=fin[:], in0=p2s[c][:], scalar=s2_ap,
                    in1=sapl[:, 8 * c:8 * c + 8, :],
                    op0=ALU.mult, op1=ALU.add,
                )
                ob = finp.tile([128, 8, 64], F32, tag="ob", name="ob")
                nc.scalar.activation(ob[:], fin[:], ACTF.Relu, bias=t2_ap)
                nc.sync.dma_start(out_d[:, 8 * c:8 * c + 8, :], ob[:])

    return nc


_CACHE = {}

# Set by test harness: run with trace=True and stash profiling info here.
TRACE = False
LAST = {}


def _get_nc():
    if "nc" not in _CACHE:
        nc = bacc.Bacc("TRN2", target_bir_lowering=False, debug=False,
                       num_devices=N_CORES)
        _build_program(nc)
        nc.compile()
        _CACHE["nc"] = nc
    return _CACHE["nc"]


def _pack_inputs(x, w1, g1, b1, w2, g2, b2, wsc, gsc, bsc):
    bf = ml_dtypes.bfloat16
    w1t = np.ascontiguousarray(
        w1.transpose(1, 2, 3, 0).reshape(256, 3200).astype(bf))
    wsct = np.ascontiguousarray(
        wsc.transpose(1, 2, 3, 0).reshape(256, 3200).astype(bf))
    w2t = np.ascontiguousarray(
        w2.transpose(1, 2, 3, 0).reshape(128, 1152).astype(bf))
    bnp = np.ascontiguousarray(
        np.stack([g1, b1, gsc, bsc, g2, b2], axis=1), dtype=np.float32)
    xb = x.astype(bf)
    in_maps = []
    for c in range(N_CORES):
        in_maps.append({
            "xs": np.ascontiguousarray(xb[c]),
            "w1t": w1t,
            "wsct": wsct,
            "w2t": w2t,
            "bnp": bnp,
        })
    return in_maps


def kernel(x, w1, g1, b1, w2, g2, b2, wsc, gsc, bsc):
    nc = _get_nc()
    in_maps = _pack_inputs(x, w1, g1, b1, w2, g2, b2, wsc, gsc, bsc)
    res = bass_utils.run_bass_kernel_spmd(
        nc, in_maps, core_ids=list(range(N_CORES)), trace=TRACE,
    )
    LAST["exec_time_ns"] = res.exec_time_ns
    LAST["results"] = res
    out = np.stack([res.results[c]["out"] for c in range(N_CORES)], axis=0)
    return out.astype(np.float32)
